# revision 1
# baseline (speedup 1.0000x reference)
"""GCN/GAT/GAT/GCN message-passing network on 8 Trainium2 NeuronCores.

Strategy (graph/data parallel, dst-partitioned):
- Nodes sharded contiguously: core r owns rows [r*6272, (r+1)*6272) (padded to 50176).
- Each layer: node-parallel transform (x @ W) computed on the owner core,
  all-gathered into a replicated DRAM "table"; edge aggregation done by the
  dst owner via dma_gather of table rows + a per-chunk one-hot matmul on the
  PE that scatter-reduces 128 edges into a 128-dst-node PSUM accumulator.
- GAT attention: softmax without max-subtraction. One-hot values are
  w_e = exp(leaky_relu(asrc[src]+adst[dst])); the normalizer z[d] comes from a
  second tiny matmul against a constant-1.0 column baked into table rows; the
  epilogue multiplies by 1/z. asrc rides the gathered row (packed column);
  adst is expanded per-edge on the DVE via indicator x broadcast + accum.
"""

import sys

sys.path.insert(0, "/opt/trn_rl_repo")

import numpy as np

import os

import concourse.bacc as bacc
import concourse.mybir as mybir
from concourse import tile
from concourse.bass_utils import run_bass_kernel_spmd
from concourse.library_config import mlp as mlp_lib

F32 = mybir.dt.float32
BF16 = mybir.dt.bfloat16
I16 = mybir.dt.int16
AL = mybir.AluOpType
ACTF = mybir.ActivationFunctionType

NCORES = 8
N, E, D, H, LOUT = 50000, 800000, 256, 256, 40
NEG = 0.2
SHARD = 6272            # 49 * 128; core 7 holds 6096 real nodes
NPAD = SHARD * NCORES   # 50176
NW = SHARD // 128       # 49 windows per core
HALF = 32768            # int16 gather index limit -> table split point
ST = 16                 # chunks per gather supertile (2048 idxs)

_BF = np.dtype(mybir.dt.np(BF16))


def _to_bf16(a):
    return np.asarray(a, np.float32).astype(_BF)


# ---------------------------------------------------------------- host prep

def preprocess(edge_index):
    """Partition edges by dst owner into 128-dst windows, split by src-half,
    pad to SPMD-uniform chunk counts. Returns (meta, per_core) where meta is
    identical for every core (defines the compiled program)."""
    src = np.asarray(edge_index[0], np.int64)
    dst = np.asarray(edge_index[1], np.int64)
    loops = np.arange(N, dtype=np.int64)
    src = np.concatenate([src, loops])
    dst = np.concatenate([dst, loops])

    deg = np.bincount(dst, minlength=N).astype(np.float64)
    dinv = 1.0 / np.sqrt(deg)
    norm = (dinv[src] * dinv[dst]).astype(np.float32)

    owner = dst // SHARD
    w_loc = (dst - owner * SHARD) // 128
    half = (src >= HALF).astype(np.int64)

    cnt = np.zeros((NCORES, NW, 2), np.int64)
    np.add.at(cnt, (owner, w_loc, half), 1)
    C = np.ceil(cnt / 128).astype(np.int64).max(axis=0)  # [NW, 2]

    CA, CB = C[:, 0], C[:, 1]
    a_off = np.concatenate([[0], np.cumsum(CA)[:-1]])
    b_off = np.concatenate([[0], np.cumsum(CB)[:-1]])
    T_A, T_B = int(CA.sum()), int(CB.sum())
    T_A_pad = -(-T_A // ST) * ST
    T_B_pad = -(-T_B // ST) * ST
    T_pad = T_A_pad + T_B_pad

    win_chunks = [
        list(range(int(a_off[w]), int(a_off[w] + CA[w])))
        + list(range(T_A_pad + int(b_off[w]), T_A_pad + int(b_off[w] + CB[w])))
        for w in range(NW)
    ]
    meta = dict(T_A_pad=T_A_pad, T_B_pad=T_B_pad, T_pad=T_pad, win_chunks=win_chunks)

    per_core = []
    for r in range(NCORES):
        sel = owner == r
        e_src, e_dst = src[sel], dst[sel]
        e_norm, e_w, e_h = norm[sel], w_loc[sel], half[sel]
        g = e_w * 2 + e_h
        order = np.argsort(g, kind="stable")
        e_src, e_dst, e_norm, e_w, e_h, g = (
            e_src[order], e_dst[order], e_norm[order], e_w[order], e_h[order], g[order])
        # position within each (w, h) group
        starts = np.searchsorted(g, np.arange(NW * 2))
        pos_in_g = np.arange(len(g)) - starts[g]
        base = np.where(e_h == 0, a_off[e_w], T_A_pad + b_off[e_w])
        chunk = base + pos_in_g // 128
        lane = pos_in_g % 128

        gidx = np.zeros((T_pad, 128), np.int16)
        dstc = np.full((T_pad, 128), 128.0, np.float32)  # sentinel kills one-hot
        valc = np.zeros((T_pad, 128), np.float32)
        gidx[chunk, lane] = (e_src - HALF * e_h).astype(np.int16)
        dstc[chunk, lane] = (e_dst % 128).astype(np.float32)
        valc[chunk, lane] = e_norm

        # wrapped gather-index layout: supertile s covers chunks [16s,16s+16);
        # flat i = c_local*128 + lane; stored at [i%16, i//16]; tiled to 128 P.
        blocks = gidx.reshape(T_pad // ST, ST * 128)
        wrapped = np.stack([b.reshape(ST * 8, 16).T for b in blocks])  # [nst,16,128]
        wrapped = np.concatenate(list(wrapped), axis=1)  # [16, T_pad*8]
        gidx_w = np.tile(wrapped, (8, 1)).astype(np.int16)

        per_core.append(dict(
            gidx=np.ascontiguousarray(gidx_w),
            dstc=np.ascontiguousarray(dstc.T),
            normc=np.ascontiguousarray(valc.T),
        ))
    return meta, per_core


def make_weight_inputs(inputs):
    """Per-core replicated weight/constant tensors."""
    W1 = np.asarray(inputs["W1"], np.float32)
    Wg = np.asarray(inputs["Wg"], np.float32)
    W2 = np.asarray(inputs["W2"], np.float32)
    a_src = np.asarray(inputs["a_src"], np.float32)
    a_dst = np.asarray(inputs["a_dst"], np.float32)
    b1 = np.asarray(inputs["b1"], np.float32)
    bg = np.asarray(inputs["bg"], np.float32)
    b2 = np.asarray(inputs["b2"], np.float32)

    Wg_ext = np.zeros((D, 384), np.float32)
    Wg_ext[:, :H] = Wg
    Wg_ext[:, 256] = Wg @ a_src
    Wg_ext[:, 257] = Wg @ a_dst
    # col 258 stays 0 in the matmul; device memsets the 1.0 afterwards
    W2_ext = np.zeros((D, 64), np.float32)
    W2_ext[:, :LOUT] = W2

    out = dict(
        W1s=_to_bf16(W1.reshape(2, 128, D)),
        Wgs=_to_bf16(Wg_ext.reshape(2, 128, 384)),
        W2s=_to_bf16(W2_ext.reshape(2, 128, 64)),
        b1b=np.ascontiguousarray(np.tile(b1, (128, 1)).astype(np.float32)),
        bgb=np.ascontiguousarray(np.tile(bg, (128, 1)).astype(np.float32)),
        b2b=np.ascontiguousarray(
            np.tile(np.pad(b2, (0, 64 - LOUT)), (128, 1)).astype(np.float32)),
        iota=np.ascontiguousarray(_to_bf16(np.tile(np.arange(128.0), (128, 1)))),
        ident=np.ascontiguousarray(_to_bf16(np.eye(128))),
        ones1=np.ascontiguousarray(_to_bf16(np.ones((1, 128)))),
    )
    return out


# ---------------------------------------------------------------- device

def build_nc(meta):
    T_pad = meta["T_pad"]
    T_A_pad = meta["T_A_pad"]
    win_chunks = meta["win_chunks"]
    n_st = T_pad // ST

    nc = bacc.Bacc("TRN2", target_bir_lowering=False)

    # -------- I/O
    xT = nc.dram_tensor("xT", [2, 128, SHARD], F32, kind="ExternalInput")
    gidx = nc.dram_tensor("gidx", [128, T_pad * 8], I16, kind="ExternalInput")
    dstc = nc.dram_tensor("dstc", [128, T_pad], F32, kind="ExternalInput")
    normc = nc.dram_tensor("normc", [128, T_pad], F32, kind="ExternalInput")
    W1s = nc.dram_tensor("W1s", [2, 128, D], BF16, kind="ExternalInput")
    Wgs = nc.dram_tensor("Wgs", [2, 128, 384], BF16, kind="ExternalInput")
    W2s = nc.dram_tensor("W2s", [2, 128, 64], BF16, kind="ExternalInput")
    b1b = nc.dram_tensor("b1b", [128, D], F32, kind="ExternalInput")
    bgb = nc.dram_tensor("bgb", [128, D], F32, kind="ExternalInput")
    b2b = nc.dram_tensor("b2b", [128, 64], F32, kind="ExternalInput")
    iota = nc.dram_tensor("iota", [128, 128], BF16, kind="ExternalInput")
    ident = nc.dram_tensor("ident", [128, 128], BF16, kind="ExternalInput")
    ones1 = nc.dram_tensor("ones1", [1, 128], BF16, kind="ExternalInput")
    out = nc.dram_tensor("out", [NW, 128, LOUT], F32, kind="ExternalOutput")

    # -------- internal DRAM
    stats_l = nc.dram_tensor("stats_l", [128, 4], F32)
    stats_g = nc.dram_tensor("stats_g", [128, 4], F32)
    sh1 = nc.dram_tensor("sh1", [NW, 128, D], BF16)
    sh2 = nc.dram_tensor("sh2", [NW, 128, 384], BF16)
    sh3 = nc.dram_tensor("sh3", [NW, 128, 384], BF16)
    sh4 = nc.dram_tensor("sh4", [NW, 128, 128], BF16)
    T1 = nc.dram_tensor("T1", [NPAD, D], BF16, addr_space="Shared")
    T2 = nc.dram_tensor("T2", [NPAD, 384], BF16, addr_space="Shared")
    T3 = nc.dram_tensor("T3", [NPAD, 384], BF16, addr_space="Shared")
    T4 = nc.dram_tensor("T4", [NPAD, 128], BF16, addr_space="Shared")
    RG = [list(range(NCORES))]

    with tile.TileContext(nc) as tc:
        with tc.tile_pool(name="persist", bufs=1) as pp:
            nc.gpsimd.load_library(mlp_lib)

            # ---- resident constants / metadata
            gidx_sb = pp.tile([128, T_pad * 8], I16, tag="gidx")
            nc.sync.dma_start(gidx_sb[:], gidx[:])
            dstc_sb = pp.tile([128, T_pad], F32, tag="dstc")
            nc.sync.dma_start(dstc_sb[:], dstc[:])
            normc_sb = pp.tile([128, T_pad], F32, tag="normc")
            nc.sync.dma_start(normc_sb[:], normc[:])
            iota_sb = pp.tile([128, 128], BF16, tag="iota")
            nc.sync.dma_start(iota_sb[:], iota[:])
            ident_sb = pp.tile([128, 128], BF16, tag="ident")
            nc.sync.dma_start(ident_sb[:], ident[:])
            ones1_sb = pp.tile([1, 128], BF16, tag="ones1")
            nc.sync.dma_start(ones1_sb[:], ones1[:])
            W1_sb = pp.tile([128, 2, D], BF16, tag="W1")
            Wg_sb = pp.tile([128, 2, 384], BF16, tag="Wg")
            W2_sb = pp.tile([128, 2, 64], BF16, tag="W2")
            for k in range(2):
                nc.sync.dma_start(W1_sb[:, k, :], W1s[k])
                nc.sync.dma_start(Wg_sb[:, k, :], Wgs[k])
                nc.sync.dma_start(W2_sb[:, k, :], W2s[k])
            b1_sb = pp.tile([128, D], F32, tag="b1")
            nc.sync.dma_start(b1_sb[:], b1b[:])
            bg_sb = pp.tile([128, D], F32, tag="bg")
            nc.sync.dma_start(bg_sb[:], bgb[:])
            b2_sb = pp.tile([128, 64], F32, tag="b2")
            nc.sync.dma_start(b2_sb[:], b2b[:])

            asm = pp.tile([128, NW, 384], BF16, tag="asm")      # table rows 1-3
            asm4 = pp.tile([128, NW, 128], BF16, tag="asm4")    # table-4 rows
            nc.vector.memset(asm4[:], 0.0)
            KSTOP = int(os.environ.get("KSTOP", "5"))
            out_asm = pp.tile([128, NW, LOUT], F32, tag="oasm")
            nc.vector.memset(out_asm[:], 0.0)

            # ================ stats + standardization params ================
            mu = pp.tile([128, 2], F32, tag="mu")
            rsd = pp.tile([128, 2], F32, tag="rsd")
            with (
                tc.tile_pool(name="xt", bufs=1) as xtp,
                tc.tile_pool(name="np1", bufs=3) as np1,
                tc.tile_pool(name="np1p", bufs=2, space="PSUM") as np1p,
            ):
                xT_sb = xtp.tile([128, 2, SHARD], F32, tag="xT")
                for k in range(2):
                    nc.sync.dma_start(xT_sb[:, k, :], xT[k])
                st_sb = xtp.tile([128, 4], F32, tag="stats")
                sq = xtp.tile([128, SHARD], F32, tag="sq")
                for k in range(2):
                    nc.vector.tensor_reduce(
                        st_sb[:, k : k + 1], xT_sb[:, k, :], mybir.AxisListType.X, AL.add)
                    nc.scalar.activation(
                        sq[:], xT_sb[:, k, :], ACTF.Square,
                        accum_out=st_sb[:, 2 + k : 3 + k])
                nc.sync.dma_start(stats_l[:], st_sb[:])
                nc.gpsimd.collective_compute(
                    "AllReduce", AL.add, replica_groups=RG,
                    ins=[stats_l[:].opt()], outs=[stats_g[:].opt()])
                stg = xtp.tile([128, 4], F32, tag="statsg")
                nc.sync.dma_start(stg[:], stats_g[:])
                # mu = sum/N ; var = (sumsq - N*mu^2)/(N-1) ; rsd = 1/sqrt(var)
                nc.vector.tensor_scalar(mu[:], stg[:, 0:2], 1.0 / N, None, AL.mult)
                mu2 = xtp.tile([128, 2], F32, tag="mu2")
                nc.vector.tensor_tensor(mu2[:], mu[:], mu[:], AL.mult)
                var = xtp.tile([128, 2], F32, tag="var")
                nc.vector.scalar_tensor_tensor(
                    var[:], mu2[:], -float(N), stg[:, 2:4], AL.mult, AL.add)
                nc.vector.tensor_scalar(var[:], var[:], 1.0 / (N - 1), None, AL.mult)
                sd = xtp.tile([128, 2], F32, tag="sd")
                nc.scalar.activation(sd[:], var[:], ACTF.Sqrt)
                nc.vector.reciprocal(rsd[:], sd[:])

                # ================ NP1: table1 = x_std @ W1 ================
                for w in range(NW):
                    ps = np1p.tile([128, D], F32, tag="ps")
                    for k in range(2):
                        xs = np1.tile([128, 128], BF16, tag="xs")
                        nc.vector.tensor_scalar(
                            xs[:], xT_sb[:, k, w * 128 : (w + 1) * 128],
                            mu[:, k : k + 1], rsd[:, k : k + 1], AL.subtract, AL.mult)
                        nc.tensor.matmul(
                            ps[:], xs[:], W1_sb[:, k, :], start=(k == 0), stop=(k == 1))
                    nc.vector.tensor_copy(asm[:, w, 0:D], ps[:])
                nc.sync.dma_start(
                    sh1[:].rearrange("w p c -> p w c"), asm[:, :, 0:D])

            nc.gpsimd.collective_compute(
                "AllGather", AL.bypass, replica_groups=RG,
                ins=[sh1[:].opt()], outs=[T1[:].opt()])

            # ================ layers ================
            def agg_layer(lidx, tbl, row_len, row_dt, gat, nl, epilogue):
                """Emit one aggregation layer. epilogue(w, psum_f, psum_z, pools)."""
                with (
                    tc.tile_pool(name=f"G{lidx}", bufs=4) as poolG,
                    tc.tile_pool(name=f"oh{lidx}", bufs=24) as poolOH,
                    tc.tile_pool(name=f"nar{lidx}", bufs=4) as poolN,
                    tc.tile_pool(name=f"adb{lidx}", bufs=1) as poolA,
                    tc.tile_pool(name=f"ep{lidx}", bufs=3) as poolE,
                    tc.tile_pool(name=f"pf{lidx}", bufs=2, space="PSUM") as poolPF,
                    tc.tile_pool(name=f"pz{lidx}", bufs=2, space="PSUM") as poolPZ,
                    tc.tile_pool(name=f"pb{lidx}", bufs=1, space="PSUM") as poolPB,
                    tc.tile_pool(name=f"pt{lidx}", bufs=1, space="PSUM") as poolPT,
                    tc.tile_pool(name=f"px{lidx}", bufs=1, space="PSUM") as poolPX,
                ):
                    G_tiles = {}

                    def get_G(st):
                        if st not in G_tiles:
                            g = poolG.tile([128, ST, row_len], row_dt, tag="G")
                            base = tbl[0:HALF, :] if st * ST < T_A_pad else tbl[HALF:NPAD, :]
                            nc.gpsimd.dma_gather(
                                g[:], base, gidx_sb[:, st * (ST * 8) : (st + 1) * (ST * 8)],
                                ST * 128, ST * 128, row_len, single_packet=False)
                            G_tiles[st] = g
                        return G_tiles[st]

                    adb_all = None
                    if gat:
                        # hoist all adst broadcasts: read table col 257 of my
                        # shard rows (in asm) BEFORE any epilogue overwrites asm
                        adb_all = poolA.tile([128, NW, 128], BF16, tag="adball")
                        for w in range(NW):
                            ptr = poolPB.tile([1, 128], BF16, tag="ptr")
                            nc.tensor.transpose(
                                ptr[:], asm[:, w, 257:258], ident_sb[:])
                            row = poolN.tile([1, 128], BF16, tag="row")
                            nc.vector.tensor_copy(row[:], ptr[:])
                            pbc = poolPB.tile([128, 128], F32, tag="pbc")
                            nc.tensor.matmul(pbc[:], ones1_sb[:], row[:])
                            nc.vector.tensor_copy(adb_all[:, w, :], pbc[:])

                    NL = nl
                    for w in range(NW):
                        chunks = win_chunks[w]
                        psf = poolPF.tile([128, NL], F32, tag="psf")
                        psz = None
                        if gat:
                            psz = poolPZ.tile([128, 1], F32, tag="psz")

                        oh_list = []
                        if gat:
                            # segments: consecutive chunks in same supertile
                            segs = []
                            for p in chunks:
                                if segs and p == segs[-1][-1] + 1 and p // ST == segs[-1][0] // ST:
                                    segs[-1].append(p)
                                else:
                                    segs.append([p])
                            for seg in segs:
                                st0 = seg[0] // ST
                                g = get_G(st0)
                                inds = []
                                tcol = poolN.tile([128, ST], F32, tag="tcol")
                                for j, p in enumerate(seg):
                                    ind = poolOH.tile([128, 128], BF16, tag="oh")
                                    nc.vector.tensor_scalar(
                                        ind[:], iota_sb[:], dstc_sb[:, p : p + 1],
                                        None, AL.is_equal)
                                    inds.append(ind)
                                    scr = poolN.tile([128, 128], BF16, tag="scr")
                                    nc.vector.scalar_tensor_tensor(
                                        scr[:], ind[:], 1.0, adb_all[:, w, :],
                                        AL.mult, AL.mult,
                                        accum_out=tcol[:, j : j + 1])
                                s0 = seg[0] % ST
                                sl = len(seg)
                                asrc_f = poolN.tile([128, ST], F32, tag="asrcf")
                                nc.vector.tensor_copy(
                                    asrc_f[:, 0:sl], g[:, s0 : s0 + sl, 256])
                                easrc = poolN.tile([128, ST], F32, tag="easrc")
                                nc.vector.tensor_tensor(
                                    easrc[:, 0:sl], asrc_f[:, 0:sl],
                                    tcol[:, 0:sl], AL.add)
                                # leaky_relu(x) = max(x, NEG*x)
                                lr = poolN.tile([128, ST], F32, tag="lr")
                                nc.vector.tensor_scalar(
                                    lr[:, 0:sl], easrc[:, 0:sl], NEG, None, AL.mult)
                                nc.vector.tensor_tensor(
                                    easrc[:, 0:sl], easrc[:, 0:sl], lr[:, 0:sl],
                                    AL.max)
                                exs = poolN.tile([128, ST], F32, tag="exs")
                                nc.scalar.activation(
                                    exs[:, 0:sl], easrc[:, 0:sl], ACTF.Exp)
                                for j, (p, ind) in enumerate(zip(seg, inds)):
                                    nc.vector.tensor_scalar(
                                        ind[:], ind[:], exs[:, j : j + 1], None,
                                        AL.mult)
                                    oh_list.append((p, ind))
                        else:
                            for p in chunks:
                                oh = poolOH.tile([128, 128], BF16, tag="oh")
                                nc.vector.tensor_scalar(
                                    oh[:], iota_sb[:], dstc_sb[:, p : p + 1],
                                    normc_sb[:, p : p + 1], AL.is_equal, AL.mult)
                                oh_list.append((p, oh))

                        n = len(oh_list)
                        for i, (p, oh) in enumerate(oh_list):
                            g = get_G(p // ST)
                            s = p % ST
                            nc.tensor.matmul(
                                psf[:], oh[:], g[:, s, 0:NL],
                                start=(i == 0), stop=(i == n - 1))
                            if gat:
                                nc.tensor.matmul(
                                    psz[:], oh[:], g[:, s, 258:259],
                                    start=(i == 0), stop=(i == n - 1))
                        epilogue(w, psf, psz, (poolE, poolPT, poolPX))

            # ---- epilogues
            def transform_store(w, h_bf, rhs_sb, ncols, dst_asm, dst_dt, pools):
                poolE, poolPT, poolPX = pools
                px = poolPX.tile([128, ncols], F32, tag="px")
                for k in range(2):
                    pt = poolPT.tile([128, 128], BF16, tag="pt")
                    nc.tensor.transpose(
                        pt[:], h_bf[:, k * 128 : (k + 1) * 128], ident_sb[:])
                    ht = poolE.tile([128, 128], BF16, tag="ht")
                    nc.vector.tensor_copy(ht[:], pt[:])
                    nc.tensor.matmul(
                        px[:], ht[:], rhs_sb[:, k, 0:ncols],
                        start=(k == 0), stop=(k == 1))
                nc.vector.tensor_copy(dst_asm, px[:])

            def epi_l1(w, psf, psz, pools):
                poolE, _, _ = pools
                hs = poolE.tile([128, D], F32, tag="hs")
                nc.vector.scalar_tensor_tensor(
                    hs[:], psf[:], 1.0, b1_sb[:], AL.mult, AL.add)
                hb = poolE.tile([128, D], BF16, tag="hb")
                nc.vector.tensor_scalar(hb[:], hs[:], 0.0, None, AL.max)
                transform_store(w, hb, Wg_sb, 384, asm[:, w, 0:384], BF16, pools)
                nc.vector.memset(asm[:, w, 258:259], 1.0)

            def epi_gat(bias_sb, rhs_sb, ncols, dst_asm_fn):
                def f(w, psf, psz, pools):
                    poolE, _, _ = pools
                    z = poolE.tile([128, 1], F32, tag="z")
                    nc.vector.tensor_scalar(z[:], psz[:], 1e-30, None, AL.add)
                    rz = poolE.tile([128, 1], F32, tag="rz")
                    nc.vector.reciprocal(rz[:], z[:])
                    hs = poolE.tile([128, D], F32, tag="hs")
                    nc.vector.scalar_tensor_tensor(
                        hs[:], psf[:], rz[:], bias_sb[:], AL.mult, AL.add)
                    hb = poolE.tile([128, D], BF16, tag="hb")
                    nc.vector.tensor_scalar(hb[:], hs[:], 0.0, None, AL.max)
                    transform_store(w, hb, rhs_sb, ncols, dst_asm_fn(w), BF16, pools)
                    if ncols == 384:
                        nc.vector.memset(asm[:, w, 258:259], 1.0)
                return f

            def epi_l4(w, psf, psz, pools):
                poolE, _, _ = pools
                lg = poolE.tile([128, 64], F32, tag="lg")
                nc.vector.scalar_tensor_tensor(
                    lg[:], psf[:], 1.0, b2_sb[:], AL.mult, AL.add)
                m = poolE.tile([128, 1], F32, tag="m")
                nc.vector.tensor_reduce(
                    m[:], lg[:, 0:LOUT], mybir.AxisListType.X, AL.max)
                negm = poolE.tile([128, 1], F32, tag="negm")
                nc.vector.tensor_scalar(negm[:], m[:], -1.0, None, AL.mult)
                es = poolE.tile([128, LOUT], F32, tag="es")
                z40 = poolE.tile([128, 1], F32, tag="z40")
                nc.scalar.activation(
                    es[:], lg[:, 0:LOUT], ACTF.Exp, bias=negm[:, 0:1],
                    accum_out=z40[:])
                lnz = poolE.tile([128, 1], F32, tag="lnz")
                nc.scalar.activation(lnz[:], z40[:], ACTF.Ln)
                nc.vector.tensor_scalar(
                    out_asm[:, w, :], lg[:, 0:LOUT], negm[:, 0:1], lnz[:, 0:1],
                    AL.add, AL.subtract)

            # L1: GCN on T1
            if KSTOP >= 2:
                agg_layer(1, T1, D, BF16, gat=False, nl=D, epilogue=epi_l1)
                nc.sync.dma_start(sh2[:].rearrange("w p c -> p w c"), asm[:])
                nc.gpsimd.collective_compute(
                    "AllGather", AL.bypass, replica_groups=RG,
                    ins=[sh2[:].opt()], outs=[T2[:].opt()])

            # L2: GAT on T2
            if KSTOP >= 3:
                agg_layer(2, T2, 384, BF16, gat=True, nl=D,
                          epilogue=epi_gat(bg_sb, Wg_sb, 384, lambda w: asm[:, w, 0:384]))
                nc.sync.dma_start(sh3[:].rearrange("w p c -> p w c"), asm[:])
                nc.gpsimd.collective_compute(
                    "AllGather", AL.bypass, replica_groups=RG,
                    ins=[sh3[:].opt()], outs=[T3[:].opt()])

            # L3: GAT on T3
            if KSTOP >= 4:
                agg_layer(3, T3, 384, BF16, gat=True, nl=D,
                          epilogue=epi_gat(bg_sb, W2_sb, 64, lambda w: asm4[:, w, 0:64]))
                nc.sync.dma_start(sh4[:].rearrange("w p c -> p w c"), asm4[:])
                nc.gpsimd.collective_compute(
                    "AllGather", AL.bypass, replica_groups=RG,
                    ins=[sh4[:].opt()], outs=[T4[:].opt()])

            # L4: GCN on T4
            if KSTOP >= 5:
                agg_layer(4, T4, 128, BF16, gat=False, nl=64, epilogue=epi_l4)
            nc.sync.dma_start(out[:].rearrange("w p c -> p w c"), out_asm[:])

    nc.compile()
    return nc


# ---------------------------------------------------------------- entry

_CACHE = {}
_RUN_KWARGS = {}


def kernel(**inputs):
    edge_index = np.asarray(inputs["edge_index"])
    key = "nc"
    if key not in _CACHE:
        meta, per_core = preprocess(edge_index)
        _CACHE["meta"] = meta
        _CACHE["per_core"] = per_core
        _CACHE[key] = build_nc(meta)
    nc = _CACHE[key]
    per_core = _CACHE["per_core"]

    wmaps = make_weight_inputs(inputs)
    x = np.asarray(inputs["x"], np.float32)
    xpad = np.zeros((NPAD, D), np.float32)
    xpad[:N] = x

    in_maps = []
    for r in range(NCORES):
        xs = xpad[r * SHARD : (r + 1) * SHARD].T  # [256, SHARD]
        m = dict(per_core[r])
        m.update(wmaps)
        m["xT"] = np.ascontiguousarray(xs.reshape(2, 128, SHARD))
        in_maps.append(m)

    res = run_bass_kernel_spmd(nc, in_maps, core_ids=list(range(NCORES)), **_RUN_KWARGS)
    _CACHE["last_res"] = res
    outs = [r["out"].reshape(SHARD, LOUT) for r in res.results]
    full = np.concatenate(outs, 0)[:N]
    return full.astype(np.float32)


if __name__ == "__main__":
    import reference

    inputs = {k: np.asarray(v) for k, v in reference.setup_inputs().items()}
    got = kernel(**inputs)
    print("kernel output", got.shape, got.dtype)



# revision 3
# speedup vs baseline: 1.1096x; 1.1096x over previous
"""GCN/GAT/GAT/GCN message-passing network on 8 Trainium2 NeuronCores. V1.

Strategy (dst-partitioned graph parallel, fp8 tables):
- Core r owns nodes [r*6272, (r+1)*6272), 49 windows of 128 dst nodes.
- Per layer: owner computes transform, AllGather into replicated fp8 DRAM
  table, dst owner aggregates via dma_gather + one-hot matmul into PSUM.
- fp8(e4m3) feature tables: T1 256B rows; T2/T3 512B rows packing
  [256 fp8 feats | fp8 1.0 | pad | bf16 asrc | pad]; T4 bf16 256B rows.
- GCN one-hot (static norm values) streamed pre-valued from DRAM (fp8),
  zero per-chunk DVE work.
- GAT: fused psf+z matmul (257 cols, ones col rides the row). Per-edge
  attention: adst via tiny PE matmul against streamed transposed indicator
  (fp8 indT), asrc rides the gathered row; w = max(exp(l), exp(0.2*l))
  batched per supertile; one DVE op per chunk builds the valued one-hot.
"""

import os
import sys

sys.path.insert(0, "/opt/trn_rl_repo")

import numpy as np

import concourse.bacc as bacc
import concourse.mybir as mybir
from concourse import tile
from concourse.bass_utils import run_bass_kernel_spmd
from concourse.library_config import mlp as mlp_lib

F32 = mybir.dt.float32
BF16 = mybir.dt.bfloat16
FP8 = mybir.dt.float8e4
I16 = mybir.dt.int16
AL = mybir.AluOpType
ACTF = mybir.ActivationFunctionType

NCORES = 8
N, E, D, H, LOUT = 50000, 800000, 256, 256, 40
NEG = 0.2
SHARD = 6272
NPAD = SHARD * NCORES
NW = SHARD // 128
HALF = 32768
ST = 16
GQ = int(os.environ.get("GQ", "1"))

_BF = np.dtype(mybir.dt.np(BF16))
_F8 = np.dtype(mybir.dt.np(FP8))


def _to_bf16(a):
    return np.asarray(a, np.float32).astype(_BF)


# ---------------------------------------------------------------- host prep

def preprocess(edge_index):
    src = np.asarray(edge_index[0], np.int64)
    dst = np.asarray(edge_index[1], np.int64)

    # degrees/norm INCLUDE self-loops (reference adds them); the loop edges
    # themselves are handled in the epilogue, not in the gathered chunks.
    deg = np.bincount(dst, minlength=N).astype(np.float64) + 1.0
    dinv = 1.0 / np.sqrt(deg)
    norm = (dinv[src] * dinv[dst]).astype(np.float32)
    slnorm_full = (dinv * dinv).astype(np.float32)  # self-edge weight per node

    owner = dst // SHARD
    w_loc = (dst - owner * SHARD) // 128
    half = (src >= HALF).astype(np.int64)

    cnt = np.zeros((NCORES, NW, 2), np.int64)
    np.add.at(cnt, (owner, w_loc, half), 1)
    C = np.ceil(cnt / 128).astype(np.int64).max(axis=0)

    CA, CB = C[:, 0], C[:, 1]
    a_off = np.concatenate([[0], np.cumsum(CA)[:-1]])
    b_off = np.concatenate([[0], np.cumsum(CB)[:-1]])
    T_A, T_B = int(CA.sum()), int(CB.sum())
    T_A_pad = -(-T_A // ST) * ST
    T_B_pad = -(-T_B // ST) * ST
    T_pad = T_A_pad + T_B_pad

    win_chunks = [
        list(range(int(a_off[w]), int(a_off[w] + CA[w])))
        + list(range(T_A_pad + int(b_off[w]), T_A_pad + int(b_off[w] + CB[w])))
        for w in range(NW)
    ]
    chunk2win = np.zeros(T_pad, np.int64)
    for w, cl in enumerate(win_chunks):
        for p in cl:
            chunk2win[p] = w
    meta = dict(T_A_pad=T_A_pad, T_B_pad=T_B_pad, T_pad=T_pad,
                win_chunks=win_chunks, chunk2win=chunk2win)

    one8 = np.float32(1.0).astype(_F8)

    # fully-padded tail chunks per half (ST rounding): mark idx -1 so the
    # gather ucode trims trailing negatives (or skips empty supertiles).
    A_end, B_end = int(CA.sum()), int(CB.sum())
    pad_chunks = list(range(A_end, T_A_pad)) + list(range(T_A_pad + B_end, T_pad))

    per_core = []
    for r in range(NCORES):
        sel = owner == r
        e_src, e_dst = src[sel], dst[sel]
        e_norm, e_w, e_h = norm[sel], w_loc[sel], half[sel]
        g = e_w * 2 + e_h
        # sort by (group, src): ascending gather addresses within each chunk
        # give the SDMA scattered reads HBM row-buffer locality.
        order = np.lexsort((e_src, g))
        e_src, e_dst, e_norm, e_w, e_h, g = (
            e_src[order], e_dst[order], e_norm[order], e_w[order], e_h[order], g[order])
        starts = np.searchsorted(g, np.arange(NW * 2))
        pos_in_g = np.arange(len(g)) - starts[g]
        base = np.where(e_h == 0, a_off[e_w], T_A_pad + b_off[e_w])
        chunk = base + pos_in_g // 128
        lane = pos_in_g % 128
        dloc = (e_dst % 128).astype(np.int64)

        gidx = np.zeros((T_pad, 128), np.int16)
        dstc = np.full((T_pad, 128), 128.0, np.float32)
        gidx[chunk, lane] = (e_src - HALF * e_h).astype(np.int16)
        dstc[chunk, lane] = dloc.astype(np.float32)

        # per-window self-loop weights for this core's dst nodes
        gids = r * SHARD + np.arange(SHARD)
        sln = np.where(gids < N, slnorm_full[np.minimum(gids, N - 1)], 0.0)
        sln = sln.reshape(NW, 128).T.astype(np.float32)  # [128, NW]

        # valued GCN one-hot stream: [lane, chunk*128 + dloc] = norm (fp8)
        ohg = np.zeros((128, T_pad * 128), _F8)
        ohg[lane, chunk * 128 + dloc] = e_norm.astype(_F8)
        # transposed indicator stream: [dloc, chunk*128 + lane] = 1.0 (fp8)
        indT = np.zeros((128, T_pad * 128), _F8)
        indT[dloc, chunk * 128 + lane] = one8

        # wrapped gather-index layout (supertiles of 16 chunks)
        blocks = gidx.reshape(T_pad // ST, ST * 128)
        wrapped = np.stack([b.reshape(ST * 8, 16).T for b in blocks])
        wrapped = np.concatenate(list(wrapped), axis=1)
        gidx_w = np.tile(wrapped, (8, 1)).astype(np.int16)

        per_core.append(dict(
            gidx=np.ascontiguousarray(gidx_w),
            dstc=np.ascontiguousarray(dstc.T),
            ohg=np.ascontiguousarray(ohg),
            indT=np.ascontiguousarray(indT),
            slnorm=np.ascontiguousarray(sln),
        ))
    return meta, per_core


def make_weight_inputs(inputs):
    W1 = np.asarray(inputs["W1"], np.float32)
    Wg = np.asarray(inputs["Wg"], np.float32)
    W2 = np.asarray(inputs["W2"], np.float32)
    a_src = np.asarray(inputs["a_src"], np.float32)
    a_dst = np.asarray(inputs["a_dst"], np.float32)
    b1 = np.asarray(inputs["b1"], np.float32)
    bg = np.asarray(inputs["bg"], np.float32)
    b2 = np.asarray(inputs["b2"], np.float32)

    Wg_ext = np.zeros((D, 258), np.float32)
    Wg_ext[:, :H] = Wg
    Wg_ext[:, 256] = Wg @ a_src
    Wg_ext[:, 257] = Wg @ a_dst
    W2_ext = np.zeros((D, 64), np.float32)
    W2_ext[:, :LOUT] = W2

    return dict(
        W1s=_to_bf16(W1.reshape(2, 128, D)),
        Wgs=_to_bf16(Wg_ext.reshape(2, 128, 258)),
        W2s=_to_bf16(W2_ext.reshape(2, 128, 64)),
        b1b=np.ascontiguousarray(np.tile(b1, (128, 1)).astype(np.float32)),
        bgb=np.ascontiguousarray(np.tile(bg, (128, 1)).astype(np.float32)),
        b2b=np.ascontiguousarray(
            np.tile(np.pad(b2, (0, 64 - LOUT)), (128, 1)).astype(np.float32)),
        iota=np.ascontiguousarray(_to_bf16(np.tile(np.arange(128.0), (128, 1)))),
        ident=np.ascontiguousarray(_to_bf16(np.eye(128))),
    )


# ---------------------------------------------------------------- device

def build_nc(meta):
    T_pad = meta["T_pad"]
    T_A_pad = meta["T_A_pad"]
    win_chunks = meta["win_chunks"]
    chunk2win = meta["chunk2win"]
    n_st = T_pad // ST

    nc = bacc.Bacc("TRN2", target_bir_lowering=False, num_swdge_queues=GQ)

    xT = nc.dram_tensor("xT", [2, 128, SHARD], F32, kind="ExternalInput")
    gidx = nc.dram_tensor("gidx", [128, T_pad * 8], I16, kind="ExternalInput")
    dstc = nc.dram_tensor("dstc", [128, T_pad], F32, kind="ExternalInput")
    ohg = nc.dram_tensor("ohg", [128, T_pad * 128], FP8, kind="ExternalInput")
    indTd = nc.dram_tensor("indT", [128, T_pad * 128], FP8, kind="ExternalInput")
    W1s = nc.dram_tensor("W1s", [2, 128, D], BF16, kind="ExternalInput")
    Wgs = nc.dram_tensor("Wgs", [2, 128, 258], BF16, kind="ExternalInput")
    W2s = nc.dram_tensor("W2s", [2, 128, 64], BF16, kind="ExternalInput")
    b1b = nc.dram_tensor("b1b", [128, D], F32, kind="ExternalInput")
    bgb = nc.dram_tensor("bgb", [128, D], F32, kind="ExternalInput")
    b2b = nc.dram_tensor("b2b", [128, 64], F32, kind="ExternalInput")
    iota = nc.dram_tensor("iota", [128, 128], BF16, kind="ExternalInput")
    ident = nc.dram_tensor("ident", [128, 128], BF16, kind="ExternalInput")
    slnormd = nc.dram_tensor("slnorm", [128, NW], F32, kind="ExternalInput")
    out = nc.dram_tensor("out", [NW, 128, LOUT], F32, kind="ExternalOutput")

    stats_l = nc.dram_tensor("stats_l", [128, 4], F32)
    stats_g = nc.dram_tensor("stats_g", [128, 4], F32)
    sh1 = nc.dram_tensor("sh1", [NW, 128, D], FP8)
    sh2 = nc.dram_tensor("sh2", [NW, 128, 512], FP8)
    sh3 = nc.dram_tensor("sh3", [NW, 128, 512], FP8)
    sh4 = nc.dram_tensor("sh4", [NW, 128, 128], BF16)
    T1 = nc.dram_tensor("T1", [NPAD, D], FP8, addr_space="Shared")
    T2 = nc.dram_tensor("T2", [NPAD, 512], FP8, addr_space="Shared")
    T3 = nc.dram_tensor("T3", [NPAD, 512], FP8, addr_space="Shared")
    T4 = nc.dram_tensor("T4", [NPAD, 128], BF16, addr_space="Shared")
    RG = [list(range(NCORES))]

    with tile.TileContext(nc) as tc:
        with tc.tile_pool(name="persist", bufs=1) as pp:
            nc.gpsimd.load_library(mlp_lib)

            gidx_sb = pp.tile([128, T_pad * 8], I16, tag="gidx")
            nc.sync.dma_start(gidx_sb[:], gidx[:])
            dstc_sb = pp.tile([128, T_pad], F32, tag="dstc")
            nc.sync.dma_start(dstc_sb[:], dstc[:])
            iota_sb = pp.tile([128, 128], BF16, tag="iota")
            nc.sync.dma_start(iota_sb[:], iota[:])
            ident_sb = pp.tile([128, 128], BF16, tag="ident")
            nc.sync.dma_start(ident_sb[:], ident[:])
            W1_sb = pp.tile([128, 2, D], BF16, tag="W1")
            Wg_sb = pp.tile([128, 2, 258], BF16, tag="Wg")
            W2_sb = pp.tile([128, 2, 64], BF16, tag="W2")
            for k in range(2):
                nc.sync.dma_start(W1_sb[:, k, :], W1s[k])
                nc.sync.dma_start(Wg_sb[:, k, :], Wgs[k])
                nc.sync.dma_start(W2_sb[:, k, :], W2s[k])
            b1_sb = pp.tile([128, D], F32, tag="b1")
            nc.sync.dma_start(b1_sb[:], b1b[:])
            bg_sb = pp.tile([128, D], F32, tag="bg")
            nc.sync.dma_start(bg_sb[:], bgb[:])
            b2_sb = pp.tile([128, 64], F32, tag="b2")
            nc.sync.dma_start(b2_sb[:], b2b[:])
            sln_sb = pp.tile([128, NW], F32, tag="sln")
            nc.sync.dma_start(sln_sb[:], slnormd[:])

            asm8 = pp.tile([128, NW, 512], FP8, tag="asm8")
            asm8_bf = asm8[:].bitcast(BF16)  # [128, NW, 256]
            nc.vector.memset(asm8[:, :, 256:257], 1.0)
            asm4 = pp.tile([128, NW, 128], BF16, tag="asm4")
            nc.vector.memset(asm4[:], 0.0)
            adstA = pp.tile([128, NW], BF16, tag="adstA")
            adstB = pp.tile([128, NW], BF16, tag="adstB")
            z_all = pp.tile([128, NW], F32, tag="zall")
            negm_all = pp.tile([128, NW], F32, tag="negm")
            out_asm = pp.tile([128, NW, LOUT], F32, tag="oasm")
            KSTOP = int(os.environ.get("KSTOP", "5"))

            # ================ stats + standardization ================
            mu = pp.tile([128, 2], F32, tag="mu")
            rsd = pp.tile([128, 2], F32, tag="rsd")
            with (
                tc.tile_pool(name="xt", bufs=1) as xtp,
                tc.tile_pool(name="np1", bufs=3) as np1,
                tc.tile_pool(name="np1p", bufs=2, space="PSUM") as np1p,
            ):
                xT_sb = xtp.tile([128, 2, SHARD], F32, tag="xT")
                for k in range(2):
                    nc.sync.dma_start(xT_sb[:, k, :], xT[k])
                st_sb = xtp.tile([128, 4], F32, tag="stats")
                sq = xtp.tile([128, SHARD], F32, tag="sq")
                for k in range(2):
                    nc.vector.tensor_reduce(
                        st_sb[:, k : k + 1], xT_sb[:, k, :], mybir.AxisListType.X, AL.add)
                    nc.scalar.activation(
                        sq[:], xT_sb[:, k, :], ACTF.Square,
                        accum_out=st_sb[:, 2 + k : 3 + k])
                nc.sync.dma_start(stats_l[:], st_sb[:])
                nc.gpsimd.collective_compute(
                    "AllReduce", AL.add, replica_groups=RG,
                    ins=[stats_l[:].opt()], outs=[stats_g[:].opt()])
                stg = xtp.tile([128, 4], F32, tag="statsg")
                nc.sync.dma_start(stg[:], stats_g[:])
                nc.vector.tensor_scalar(mu[:], stg[:, 0:2], 1.0 / N, None, AL.mult)
                mu2 = xtp.tile([128, 2], F32, tag="mu2")
                nc.vector.tensor_tensor(mu2[:], mu[:], mu[:], AL.mult)
                var = xtp.tile([128, 2], F32, tag="var")
                nc.vector.scalar_tensor_tensor(
                    var[:], mu2[:], -float(N), stg[:, 2:4], AL.mult, AL.add)
                nc.vector.tensor_scalar(var[:], var[:], 1.0 / (N - 1), None, AL.mult)
                sd = xtp.tile([128, 2], F32, tag="sd")
                nc.scalar.activation(sd[:], var[:], ACTF.Sqrt)
                nc.vector.reciprocal(rsd[:], sd[:])

                # ================ NP1: T1 = x_std @ W1 (fp8) ================
                for w in range(NW):
                    ps = np1p.tile([128, D], F32, tag="ps")
                    for k in range(2):
                        xs = np1.tile([128, 128], BF16, tag="xs")
                        nc.vector.tensor_scalar(
                            xs[:], xT_sb[:, k, w * 128 : (w + 1) * 128],
                            mu[:, k : k + 1], rsd[:, k : k + 1], AL.subtract, AL.mult)
                        nc.tensor.matmul(
                            ps[:], xs[:], W1_sb[:, k, :], start=(k == 0), stop=(k == 1))
                    nc.vector.tensor_copy(asm8[:, w, 0:D], ps[:])
                nc.sync.dma_start(
                    sh1[:].rearrange("w p c -> p w c"), asm8[:, :, 0:D])

            nc.gpsimd.collective_compute(
                "AllGather", AL.bypass, replica_groups=RG,
                ins=[sh1[:].opt()], outs=[T1[:].opt()])

            # ================ aggregation layers ================
            def agg_layer(lidx, tbl, row_b, row_dt, gat, nl, adst_sb, epilogue):
                """row_b = row bytes; nl = psf cols fed to epilogue."""
                elem = row_b // mybir.dt.size(row_dt)
                mmcols = 257 if gat else nl
                with (
                    tc.tile_pool(name=f"G{lidx}", bufs=4) as poolG,
                    tc.tile_pool(name=f"S{lidx}", bufs=4) as poolS,
                    tc.tile_pool(name=f"w{lidx}", bufs=4) as poolW,
                    tc.tile_pool(name=f"oh{lidx}", bufs=12) as poolOH,
                    tc.tile_pool(name=f"n{lidx}", bufs=3) as poolN,
                    tc.tile_pool(name=f"ep{lidx}", bufs=3) as poolE,
                    tc.tile_pool(name=f"pf{lidx}", bufs=2, space="PSUM") as poolPF,
                    tc.tile_pool(name=f"pc{lidx}", bufs=2, space="PSUM") as poolPC,
                    tc.tile_pool(name=f"px{lidx}", bufs=1, space="PSUM") as poolPX,
                    tc.tile_pool(name=f"pt{lidx}", bufs=2, space="PSUM") as poolPT,
                ):
                    G_tiles = {}
                    S_tiles = {}
                    W_tiles = {}

                    def get_G(st):
                        if st not in G_tiles:
                            g = poolG.tile([128, ST, elem], row_dt, tag="G")
                            base = (tbl[0:HALF, :] if st * ST < T_A_pad
                                    else tbl[HALF:NPAD, :])
                            nc.gpsimd.dma_gather(
                                g[:], base,
                                gidx_sb[:, st * (ST * 8) : (st + 1) * (ST * 8)],
                                ST * 128, ST * 128, elem,
                                single_packet=False, queue_num=st % GQ)
                            G_tiles[st] = g
                        return G_tiles[st]

                    def get_OHS(st):
                        if st not in S_tiles:
                            s = poolS.tile([128, ST * 128], FP8, tag="ohs")
                            nc.sync.dma_start(
                                s[:], ohg[:, st * ST * 128 : (st + 1) * ST * 128])
                            S_tiles[st] = s
                        return S_tiles[st]

                    def get_W(st):
                        if st not in W_tiles:
                            it = poolS.tile([128, ST * 128], FP8, tag="indt")
                            nc.sync.dma_start(
                                it[:], indTd[:, st * ST * 128 : (st + 1) * ST * 128])
                            ptc = poolPC.tile([128, ST], F32, tag="ptc")
                            for j in range(ST):
                                p = st * ST + j
                                w = int(chunk2win[p])
                                nc.tensor.matmul(
                                    ptc[:, j : j + 1],
                                    it[:, j * 128 : (j + 1) * 128],
                                    adst_sb[:, w : w + 1], start=True, stop=True)
                            g = get_G(st)
                            gb = g[:].bitcast(BF16)
                            tcs = poolN.tile([128, ST], F32, tag="tcs")
                            nc.scalar.activation(tcs[:], ptc[:], ACTF.Copy)
                            asr = poolN.tile([128, ST], F32, tag="asr")
                            nc.scalar.activation(asr[:], gb[:, :, 129], ACTF.Copy)
                            ell = poolN.tile([128, ST], F32, tag="ell")
                            nc.vector.tensor_tensor(ell[:], tcs[:], asr[:], AL.add)
                            e1 = poolN.tile([128, ST], BF16, tag="e1")
                            nc.scalar.activation(e1[:], ell[:], ACTF.Exp)
                            e5 = poolN.tile([128, ST], BF16, tag="e5")
                            nc.scalar.activation(e5[:], ell[:], ACTF.Exp, scale=NEG)
                            ws = poolW.tile([128, ST], F32, tag="ws")
                            nc.vector.tensor_tensor(ws[:], e1[:], e5[:], AL.max)
                            W_tiles[st] = ws
                        return W_tiles[st]

                    for w in range(NW):
                        chunks = win_chunks[w]
                        psf = poolPF.tile([128, mmcols], F32, tag="psf")
                        n = len(chunks)
                        if n == 0:
                            zps = poolE.tile([128, mmcols], F32, tag="zps")
                            nc.vector.memset(zps[:], 0.0)
                            epilogue(w, zps, (poolE, poolPX, poolPT))
                            continue
                        for i, p in enumerate(chunks):
                            st, j = p // ST, p % ST
                            g = get_G(st)
                            if gat:
                                ws = get_W(st)
                                oh = poolOH.tile([128, 128], BF16, tag="oh")
                                nc.vector.tensor_scalar(
                                    oh[:], iota_sb[:], dstc_sb[:, p : p + 1],
                                    ws[:, j : j + 1], AL.is_equal, AL.mult)
                            else:
                                ohs = get_OHS(st)
                                oh = ohs[:, j * 128 : (j + 1) * 128]
                            nc.tensor.matmul(
                                psf[:], oh if gat is False else oh[:],
                                g[:, j, 0:mmcols],
                                start=(i == 0), stop=(i == n - 1))
                        epilogue(w, psf, (poolE, poolPX, poolPT))

            # ---- epilogue helpers
            def transform_store(w, hb, rhs_sb, ncols, pools, store):
                poolE, poolPX, poolPT = pools
                px = poolPX.tile([128, ncols], F32, tag="px")
                for k in range(2):
                    pt = poolPT.tile([128, 128], BF16, tag="pt")
                    nc.tensor.transpose(
                        pt[:], hb[:, k * 128 : (k + 1) * 128], ident_sb[:])
                    ht = poolE.tile([128, 128], BF16, tag="ht")
                    nc.scalar.activation(ht[:], pt[:], ACTF.Copy)
                    nc.tensor.matmul(
                        px[:], ht[:], rhs_sb[:, k, 0:ncols],
                        start=(k == 0), stop=(k == 1))
                store(w, px)

            def store_gat(adst_next):
                def f(w, px):
                    nc.vector.tensor_copy(asm8[:, w, 0:256], px[:, 0:256])
                    nc.scalar.activation(
                        asm8_bf[:, w, 129:130], px[:, 256:257], ACTF.Copy)
                    nc.scalar.activation(
                        adst_next[:, w : w + 1], px[:, 257:258], ACTF.Copy)
                return f

            def store_l4(w, px):
                nc.scalar.activation(asm4[:, w, 0:64], px[:, 0:64], ACTF.Copy)

            def epi_l1(w, psf, pools):
                poolE, _, _ = pools
                hs = poolE.tile([128, D], F32, tag="hs")
                nc.vector.scalar_tensor_tensor(
                    hs[:], psf[:, 0:D], 1.0, b1_sb[:], AL.mult, AL.add)
                # self-loop: hs += slnorm[d] * (own T1 row)
                nc.vector.scalar_tensor_tensor(
                    hs[:], asm8[:, w, 0:D], sln_sb[:, w : w + 1],
                    hs[:], AL.mult, AL.add)
                hb = poolE.tile([128, D], BF16, tag="hb")
                nc.scalar.activation(hb[:], hs[:], ACTF.Relu)
                transform_store(w, hb, Wg_sb, 258, pools, store_gat(adstA))

            def epi_gat(bias_sb, rhs_sb, ncols, store, adst_cur):
                def f(w, psf, pools):
                    poolE, _, _ = pools
                    # self-loop: w_self = exp(lrelu(asrc_d + adst_d))
                    lsf = poolE.tile([128, 1], F32, tag="lsf")
                    nc.vector.tensor_tensor(
                        lsf[:], asm8_bf[:, w, 129:130],
                        adst_cur[:, w : w + 1], AL.add)
                    e1s = poolE.tile([128, 1], F32, tag="e1s")
                    nc.scalar.activation(e1s[:], lsf[:], ACTF.Exp)
                    e5s = poolE.tile([128, 1], F32, tag="e5s")
                    nc.scalar.activation(e5s[:], lsf[:], ACTF.Exp, scale=NEG)
                    wse = poolE.tile([128, 1], F32, tag="wse")
                    nc.vector.tensor_tensor(wse[:], e1s[:], e5s[:], AL.max)
                    # z includes the self edge; z > 0 guaranteed
                    zt = poolE.tile([128, 1], F32, tag="zt")
                    nc.vector.tensor_scalar(
                        zt[:], psf[:, 256:257], 1e-30, None, AL.add)
                    zt2 = poolE.tile([128, 1], F32, tag="zt2")
                    nc.vector.tensor_tensor(zt2[:], zt[:], wse[:], AL.add)
                    rz = poolE.tile([128, 1], F32, tag="rz")
                    nc.vector.reciprocal(rz[:], zt2[:])
                    wrz = poolE.tile([128, 1], F32, tag="wrz")
                    nc.vector.tensor_tensor(wrz[:], wse[:], rz[:], AL.mult)
                    hs = poolE.tile([128, D], F32, tag="hs")
                    nc.vector.scalar_tensor_tensor(
                        hs[:], psf[:, 0:D], rz[:], bias_sb[:], AL.mult, AL.add)
                    # hs += (w_self/z) * own row
                    nc.vector.scalar_tensor_tensor(
                        hs[:], asm8[:, w, 0:D], wrz[:], hs[:], AL.mult, AL.add)
                    hb = poolE.tile([128, D], BF16, tag="hb")
                    nc.scalar.activation(hb[:], hs[:], ACTF.Relu)
                    transform_store(w, hb, rhs_sb, ncols, pools, store)
                return f

            def epi_l4(w, psf, pools):
                poolE, _, _ = pools
                lg = poolE.tile([128, 64], F32, tag="lg")
                nc.vector.scalar_tensor_tensor(
                    lg[:], psf[:, 0:64], 1.0, b2_sb[:], AL.mult, AL.add)
                nc.vector.scalar_tensor_tensor(
                    lg[:], asm4[:, w, 0:64], sln_sb[:, w : w + 1],
                    lg[:], AL.mult, AL.add)
                m = poolE.tile([128, 1], F32, tag="m")
                nc.vector.tensor_reduce(
                    m[:], lg[:, 0:LOUT], mybir.AxisListType.X, AL.max)
                nc.vector.tensor_scalar(
                    negm_all[:, w : w + 1], m[:], -1.0, None, AL.mult)
                es = poolE.tile([128, LOUT], F32, tag="es")
                nc.scalar.activation(
                    es[:], lg[:, 0:LOUT], ACTF.Exp,
                    bias=negm_all[:, w : w + 1], accum_out=z_all[:, w : w + 1])
                nc.vector.tensor_scalar(
                    out_asm[:, w, :], lg[:, 0:LOUT], negm_all[:, w : w + 1],
                    None, AL.add)

            # L1: GCN on T1
            if KSTOP >= 2:
                agg_layer(1, T1, 256, FP8, gat=False, nl=D, adst_sb=None,
                          epilogue=epi_l1)
                nc.sync.dma_start(sh2[:].rearrange("w p c -> p w c"), asm8[:])
                nc.gpsimd.collective_compute(
                    "AllGather", AL.bypass, replica_groups=RG,
                    ins=[sh2[:].opt()], outs=[T2[:].opt()])

            # L2: GAT on T2
            if KSTOP >= 3:
                agg_layer(2, T2, 512, FP8, gat=True, nl=D, adst_sb=adstA,
                          epilogue=epi_gat(bg_sb, Wg_sb, 258, store_gat(adstB), adstA))
                nc.sync.dma_start(sh3[:].rearrange("w p c -> p w c"), asm8[:])
                nc.gpsimd.collective_compute(
                    "AllGather", AL.bypass, replica_groups=RG,
                    ins=[sh3[:].opt()], outs=[T3[:].opt()])

            # L3: GAT on T3
            if KSTOP >= 4:
                agg_layer(3, T3, 512, FP8, gat=True, nl=64, adst_sb=adstB,
                          epilogue=epi_gat(bg_sb, W2_sb, 64, store_l4, adstB))
                nc.sync.dma_start(sh4[:].rearrange("w p c -> p w c"), asm4[:])
                nc.gpsimd.collective_compute(
                    "AllGather", AL.bypass, replica_groups=RG,
                    ins=[sh4[:].opt()], outs=[T4[:].opt()])

            # L4: GCN on T4
            if KSTOP >= 5:
                agg_layer(4, T4, 256, BF16, gat=False, nl=64, adst_sb=None,
                          epilogue=epi_l4)
                lnz = pp.tile([128, NW], F32, tag="lnz")
                nc.scalar.activation(lnz[:], z_all[:], ACTF.Ln)
                for w in range(NW):
                    nc.vector.tensor_scalar(
                        out_asm[:, w, :], out_asm[:, w, :],
                        lnz[:, w : w + 1], None, AL.subtract)
            nc.sync.dma_start(out[:].rearrange("w p c -> p w c"), out_asm[:])

    nc.compile()
    return nc


# ---------------------------------------------------------------- entry

_CACHE = {}
_RUN_KWARGS = {}


def kernel(**inputs):
    edge_index = np.asarray(inputs["edge_index"])
    if "nc" not in _CACHE:
        meta, per_core = preprocess(edge_index)
        _CACHE["meta"] = meta
        _CACHE["per_core"] = per_core
        _CACHE["nc"] = build_nc(meta)
    nc = _CACHE["nc"]
    per_core = _CACHE["per_core"]

    wmaps = make_weight_inputs(inputs)
    x = np.asarray(inputs["x"], np.float32)
    xpad = np.zeros((NPAD, D), np.float32)
    xpad[:N] = x

    in_maps = []
    for r in range(NCORES):
        xs = xpad[r * SHARD : (r + 1) * SHARD].T
        m = dict(per_core[r])
        m.update(wmaps)
        m["xT"] = np.ascontiguousarray(xs.reshape(2, 128, SHARD))
        in_maps.append(m)

    res = run_bass_kernel_spmd(nc, in_maps, core_ids=list(range(NCORES)), **_RUN_KWARGS)
    _CACHE["last_res"] = res
    outs = [r["out"].reshape(SHARD, LOUT) for r in res.results]
    full = np.concatenate(outs, 0)[:N]
    return full.astype(np.float32)


if __name__ == "__main__":
    import reference

    inputs = {k: np.asarray(v) for k, v in reference.setup_inputs().items()}
    got = kernel(**inputs)
    print("kernel output", got.shape, got.dtype)


# revision 4
# speedup vs baseline: 1.1392x; 1.0267x over previous
"""GCN/GAT/GAT/GCN message-passing network on 8 Trainium2 NeuronCores. V1.

Strategy (dst-partitioned graph parallel, fp8 tables):
- Core r owns nodes [r*6272, (r+1)*6272), 49 windows of 128 dst nodes.
- Per layer: owner computes transform, AllGather into replicated fp8 DRAM
  table, dst owner aggregates via dma_gather + one-hot matmul into PSUM.
- fp8(e4m3) feature tables: T1 256B rows; T2/T3 512B rows packing
  [256 fp8 feats | fp8 1.0 | pad | bf16 asrc | pad]; T4 bf16 256B rows.
- GCN one-hot (static norm values) streamed pre-valued from DRAM (fp8),
  zero per-chunk DVE work.
- GAT: fused psf+z matmul (257 cols, ones col rides the row). Per-edge
  attention: adst via tiny PE matmul against streamed transposed indicator
  (fp8 indT), asrc rides the gathered row; w = max(exp(l), exp(0.2*l))
  batched per supertile; one DVE op per chunk builds the valued one-hot.
"""

import os
import sys

sys.path.insert(0, "/opt/trn_rl_repo")

import numpy as np

import concourse.bacc as bacc
import concourse.mybir as mybir
from concourse import tile
from concourse.bass_utils import run_bass_kernel_spmd
from concourse.library_config import mlp as mlp_lib

F32 = mybir.dt.float32
BF16 = mybir.dt.bfloat16
FP8 = mybir.dt.float8e4
I16 = mybir.dt.int16
AL = mybir.AluOpType
ACTF = mybir.ActivationFunctionType

NCORES = 8
N, E, D, H, LOUT = 50000, 800000, 256, 256, 40
NEG = 0.2
SHARD = 6272
NPAD = SHARD * NCORES
NW = SHARD // 128
HALF = 32768
ST = 16
GQ = int(os.environ.get("GQ", "1"))

_BF = np.dtype(mybir.dt.np(BF16))
_F8 = np.dtype(mybir.dt.np(FP8))


def _to_bf16(a):
    return np.asarray(a, np.float32).astype(_BF)


# ---------------------------------------------------------------- host prep

def preprocess(edge_index):
    src = np.asarray(edge_index[0], np.int64)
    dst = np.asarray(edge_index[1], np.int64)

    # degrees/norm INCLUDE self-loops (reference adds them); the loop edges
    # themselves are handled in the epilogue, not in the gathered chunks.
    deg = np.bincount(dst, minlength=N).astype(np.float64) + 1.0
    dinv = 1.0 / np.sqrt(deg)
    norm = (dinv[src] * dinv[dst]).astype(np.float32)
    slnorm_full = (dinv * dinv).astype(np.float32)  # self-edge weight per node
    # total in-norm per dst (incl self) for the standardization folding
    sfull_full = (np.bincount(dst, weights=norm.astype(np.float64), minlength=N)
                  .astype(np.float32) + slnorm_full)

    owner = dst // SHARD
    w_loc = (dst - owner * SHARD) // 128
    half = (src >= HALF).astype(np.int64)

    cnt = np.zeros((NCORES, NW, 2), np.int64)
    np.add.at(cnt, (owner, w_loc, half), 1)
    C = np.ceil(cnt / 128).astype(np.int64).max(axis=0)

    CA, CB = C[:, 0], C[:, 1]
    a_off = np.concatenate([[0], np.cumsum(CA)[:-1]])
    b_off = np.concatenate([[0], np.cumsum(CB)[:-1]])
    T_A, T_B = int(CA.sum()), int(CB.sum())
    T_A_pad = -(-T_A // ST) * ST
    T_B_pad = -(-T_B // ST) * ST
    T_pad = T_A_pad + T_B_pad

    win_chunks = [
        list(range(int(a_off[w]), int(a_off[w] + CA[w])))
        + list(range(T_A_pad + int(b_off[w]), T_A_pad + int(b_off[w] + CB[w])))
        for w in range(NW)
    ]
    chunk2win = np.zeros(T_pad, np.int64)
    for w, cl in enumerate(win_chunks):
        for p in cl:
            chunk2win[p] = w
    meta = dict(T_A_pad=T_A_pad, T_B_pad=T_B_pad, T_pad=T_pad,
                win_chunks=win_chunks, chunk2win=chunk2win)

    one8 = np.float32(1.0).astype(_F8)

    # fully-padded tail chunks per half (ST rounding): mark idx -1 so the
    # gather ucode trims trailing negatives (or skips empty supertiles).
    A_end, B_end = int(CA.sum()), int(CB.sum())
    pad_chunks = list(range(A_end, T_A_pad)) + list(range(T_A_pad + B_end, T_pad))

    per_core = []
    for r in range(NCORES):
        sel = owner == r
        e_src, e_dst = src[sel], dst[sel]
        e_norm, e_w, e_h = norm[sel], w_loc[sel], half[sel]
        g = e_w * 2 + e_h
        # sort by (group, src): ascending gather addresses within each chunk
        # give the SDMA scattered reads HBM row-buffer locality.
        order = np.lexsort((e_src, g))
        e_src, e_dst, e_norm, e_w, e_h, g = (
            e_src[order], e_dst[order], e_norm[order], e_w[order], e_h[order], g[order])
        starts = np.searchsorted(g, np.arange(NW * 2))
        pos_in_g = np.arange(len(g)) - starts[g]
        base = np.where(e_h == 0, a_off[e_w], T_A_pad + b_off[e_w])
        chunk = base + pos_in_g // 128
        lane = pos_in_g % 128
        dloc = (e_dst % 128).astype(np.int64)

        gidx = np.zeros((T_pad, 128), np.int16)
        dstc = np.full((T_pad, 128), 128.0, np.float32)
        gidx[chunk, lane] = (e_src - HALF * e_h).astype(np.int16)
        dstc[chunk, lane] = dloc.astype(np.float32)

        # per-window self-loop weights for this core's dst nodes
        gids = r * SHARD + np.arange(SHARD)
        sln = np.where(gids < N, slnorm_full[np.minimum(gids, N - 1)], 0.0)
        sln = sln.reshape(NW, 128).T.astype(np.float32)  # [128, NW]
        sfl = np.where(gids < N, sfull_full[np.minimum(gids, N - 1)], 0.0)
        sfl = sfl.reshape(NW, 128).T.astype(np.float32)  # [128, NW]

        # valued GCN one-hot stream: [lane, chunk*128 + dloc] = norm (fp8)
        ohg = np.zeros((128, T_pad * 128), _F8)
        ohg[lane, chunk * 128 + dloc] = e_norm.astype(_F8)
        # transposed indicator stream: [dloc, chunk*128 + lane] = 1.0 (fp8)
        indT = np.zeros((128, T_pad * 128), _F8)
        indT[dloc, chunk * 128 + lane] = one8

        # wrapped gather-index layout (supertiles of 16 chunks)
        blocks = gidx.reshape(T_pad // ST, ST * 128)
        wrapped = np.stack([b.reshape(ST * 8, 16).T for b in blocks])
        wrapped = np.concatenate(list(wrapped), axis=1)
        gidx_w = np.tile(wrapped, (8, 1)).astype(np.int16)

        per_core.append(dict(
            gidx=np.ascontiguousarray(gidx_w),
            dstc=np.ascontiguousarray(dstc.T),
            ohg=np.ascontiguousarray(ohg),
            indT=np.ascontiguousarray(indT),
            slnorm=np.ascontiguousarray(sln),
            sfull=np.ascontiguousarray(-sfl),
        ))
    return meta, per_core


def make_weight_inputs(inputs):
    W1 = np.asarray(inputs["W1"], np.float32)
    Wg = np.asarray(inputs["Wg"], np.float32)
    W2 = np.asarray(inputs["W2"], np.float32)
    a_src = np.asarray(inputs["a_src"], np.float32)
    a_dst = np.asarray(inputs["a_dst"], np.float32)
    b1 = np.asarray(inputs["b1"], np.float32)
    bg = np.asarray(inputs["bg"], np.float32)
    b2 = np.asarray(inputs["b2"], np.float32)

    Wg_ext = np.zeros((D, 258), np.float32)
    Wg_ext[:, :H] = Wg
    Wg_ext[:, 256] = Wg @ a_src
    Wg_ext[:, 257] = Wg @ a_dst
    W2_ext = np.zeros((D, 64), np.float32)
    W2_ext[:, :LOUT] = W2

    return dict(
        W1s=_to_bf16(W1.reshape(2, 128, D)),
        Wgs=_to_bf16(Wg_ext.reshape(2, 128, 258)),
        W2s=_to_bf16(W2_ext.reshape(2, 128, 64)),
        b1b=np.ascontiguousarray(np.tile(b1, (128, 1)).astype(np.float32)),
        bgb=np.ascontiguousarray(np.tile(bg, (128, 1)).astype(np.float32)),
        b2b=np.ascontiguousarray(
            np.tile(np.pad(b2, (0, 64 - LOUT)), (128, 1)).astype(np.float32)),
        iota=np.ascontiguousarray(_to_bf16(np.tile(np.arange(128.0), (128, 1)))),
        ident=np.ascontiguousarray(_to_bf16(np.eye(128))),
    )


# ---------------------------------------------------------------- device

def build_nc(meta):
    T_pad = meta["T_pad"]
    T_A_pad = meta["T_A_pad"]
    win_chunks = meta["win_chunks"]
    chunk2win = meta["chunk2win"]
    n_st = T_pad // ST

    nc = bacc.Bacc("TRN2", target_bir_lowering=False, num_swdge_queues=GQ)

    xT = nc.dram_tensor("xT", [2, 128, SHARD], F32, kind="ExternalInput")
    gidx = nc.dram_tensor("gidx", [128, T_pad * 8], I16, kind="ExternalInput")
    dstc = nc.dram_tensor("dstc", [128, T_pad], F32, kind="ExternalInput")
    ohg = nc.dram_tensor("ohg", [128, T_pad * 128], FP8, kind="ExternalInput")
    indTd = nc.dram_tensor("indT", [128, T_pad * 128], FP8, kind="ExternalInput")
    W1s = nc.dram_tensor("W1s", [2, 128, D], BF16, kind="ExternalInput")
    Wgs = nc.dram_tensor("Wgs", [2, 128, 258], BF16, kind="ExternalInput")
    W2s = nc.dram_tensor("W2s", [2, 128, 64], BF16, kind="ExternalInput")
    b1b = nc.dram_tensor("b1b", [128, D], F32, kind="ExternalInput")
    bgb = nc.dram_tensor("bgb", [128, D], F32, kind="ExternalInput")
    b2b = nc.dram_tensor("b2b", [128, 64], F32, kind="ExternalInput")
    iota = nc.dram_tensor("iota", [128, 128], BF16, kind="ExternalInput")
    ident = nc.dram_tensor("ident", [128, 128], BF16, kind="ExternalInput")
    slnormd = nc.dram_tensor("slnorm", [128, NW], F32, kind="ExternalInput")
    sfulld = nc.dram_tensor("sfull", [128, NW], F32, kind="ExternalInput")
    Xtbl = nc.dram_tensor("Xtbl", [NPAD, D], FP8, kind="ExternalInput")
    xown = nc.dram_tensor("xown", [128, NW, D], FP8, kind="ExternalInput")
    out = nc.dram_tensor("out", [NW, 128, LOUT], F32, kind="ExternalOutput")

    stats_l = nc.dram_tensor("stats_l", [128, 4], F32)
    mr_dram = nc.dram_tensor("mr_dram", [4, 128], F32)
    stats_g = nc.dram_tensor("stats_g", [128, 4], F32)
    sh1 = nc.dram_tensor("sh1", [NW, 128, D], FP8)
    sh2 = nc.dram_tensor("sh2", [NW, 128, 512], FP8)
    sh3 = nc.dram_tensor("sh3", [NW, 128, 512], FP8)
    sh4 = nc.dram_tensor("sh4", [NW, 128, 128], BF16)
    T1 = nc.dram_tensor("T1", [NPAD, D], FP8, addr_space="Shared")
    T2 = nc.dram_tensor("T2", [NPAD, 512], FP8, addr_space="Shared")
    T3 = nc.dram_tensor("T3", [NPAD, 512], FP8, addr_space="Shared")
    T4 = nc.dram_tensor("T4", [NPAD, 128], BF16, addr_space="Shared")
    RG = [list(range(NCORES))]

    with tile.TileContext(nc) as tc:
        with tc.tile_pool(name="persist", bufs=1) as pp:
            nc.gpsimd.load_library(mlp_lib)

            gidx_sb = pp.tile([128, T_pad * 8], I16, tag="gidx")
            nc.sync.dma_start(gidx_sb[:], gidx[:])
            dstc_sb = pp.tile([128, T_pad], F32, tag="dstc")
            nc.sync.dma_start(dstc_sb[:], dstc[:])
            iota_sb = pp.tile([128, 128], BF16, tag="iota")
            nc.sync.dma_start(iota_sb[:], iota[:])
            ident_sb = pp.tile([128, 128], BF16, tag="ident")
            nc.sync.dma_start(ident_sb[:], ident[:])
            W1_sb = pp.tile([128, 2, D], BF16, tag="W1")
            Wg_sb = pp.tile([128, 2, 258], BF16, tag="Wg")
            W2_sb = pp.tile([128, 2, 64], BF16, tag="W2")
            for k in range(2):
                nc.sync.dma_start(W1_sb[:, k, :], W1s[k])
                nc.sync.dma_start(Wg_sb[:, k, :], Wgs[k])
                nc.sync.dma_start(W2_sb[:, k, :], W2s[k])
            b1_sb = pp.tile([128, D], F32, tag="b1")
            nc.sync.dma_start(b1_sb[:], b1b[:])
            bg_sb = pp.tile([128, D], F32, tag="bg")
            nc.sync.dma_start(bg_sb[:], bgb[:])
            b2_sb = pp.tile([128, 64], F32, tag="b2")
            nc.sync.dma_start(b2_sb[:], b2b[:])
            sln_sb = pp.tile([128, NW], F32, tag="sln")
            nc.sync.dma_start(sln_sb[:], slnormd[:])
            sfl_sb = pp.tile([128, NW], F32, tag="sfl")
            nc.sync.dma_start(sfl_sb[:], sfulld[:])
            xown_sb = pp.tile([128, NW, D], FP8, tag="xown")
            nc.sync.dma_start(xown_sb[:], xown[:])
            onesT2 = pp.tile([4, 128], BF16, tag="onesT2")
            nc.vector.memset(onesT2[:], 1.0)
            rsd_bc = pp.tile([128, D], F32, tag="rsdbc")
            mrs_bc = pp.tile([128, D], F32, tag="mrsbc")

            asm8 = pp.tile([128, NW, 512], FP8, tag="asm8")
            asm8_bf = asm8[:].bitcast(BF16)  # [128, NW, 256]
            nc.vector.memset(asm8[:, :, 256:257], 1.0)
            asm4 = pp.tile([128, NW, 128], BF16, tag="asm4")
            nc.vector.memset(asm4[:], 0.0)
            adstA = pp.tile([128, NW], BF16, tag="adstA")
            adstB = pp.tile([128, NW], BF16, tag="adstB")
            z_all = pp.tile([128, NW], F32, tag="zall")
            negm_all = pp.tile([128, NW], F32, tag="negm")
            out_asm = pp.tile([128, NW, LOUT], F32, tag="oasm")
            KSTOP = int(os.environ.get("KSTOP", "5"))

            # ================ stats + standardization ================
            mu = pp.tile([128, 2], F32, tag="mu")
            rsd = pp.tile([128, 2], F32, tag="rsd")
            with (
                tc.tile_pool(name="xt", bufs=1) as xtp,
                tc.tile_pool(name="np1", bufs=3) as np1,
                tc.tile_pool(name="np1p", bufs=2, space="PSUM") as np1p,
            ):
                xT_sb = xtp.tile([128, 2, SHARD], F32, tag="xT")
                for k in range(2):
                    nc.sync.dma_start(xT_sb[:, k, :], xT[k])
                st_sb = xtp.tile([128, 4], F32, tag="stats")
                sq = xtp.tile([128, SHARD], F32, tag="sq")
                for k in range(2):
                    nc.vector.tensor_reduce(
                        st_sb[:, k : k + 1], xT_sb[:, k, :], mybir.AxisListType.X, AL.add)
                    nc.scalar.activation(
                        sq[:], xT_sb[:, k, :], ACTF.Square,
                        accum_out=st_sb[:, 2 + k : 3 + k])
                nc.sync.dma_start(stats_l[:], st_sb[:])
                nc.gpsimd.collective_compute(
                    "AllReduce", AL.add, replica_groups=RG,
                    ins=[stats_l[:].opt()], outs=[stats_g[:].opt()])
                stg = xtp.tile([128, 4], F32, tag="statsg")
                nc.sync.dma_start(stg[:], stats_g[:])
                nc.vector.tensor_scalar(mu[:], stg[:, 0:2], 1.0 / N, None, AL.mult)
                mu2 = xtp.tile([128, 2], F32, tag="mu2")
                nc.vector.tensor_tensor(mu2[:], mu[:], mu[:], AL.mult)
                var = xtp.tile([128, 2], F32, tag="var")
                nc.vector.scalar_tensor_tensor(
                    var[:], mu2[:], -float(N), stg[:, 2:4], AL.mult, AL.add)
                nc.vector.tensor_scalar(var[:], var[:], 1.0 / (N - 1), None, AL.mult)
                sd = xtp.tile([128, 2], F32, tag="sd")
                nc.scalar.activation(sd[:], var[:], ACTF.Sqrt)
                nc.vector.reciprocal(rsd[:], sd[:])

                # ==== broadcast mu/rsd along the free axis (exact f32) ====
                mr4 = np1.tile([128, 4], F32, tag="mr4")
                nc.vector.tensor_copy(mr4[:, 0:2], mu[:])
                nc.vector.tensor_copy(mr4[:, 2:4], rsd[:])
                nc.sync.dma_start(mr_dram[:].rearrange("k p -> p k"), mr4[:])
                mrow = np1.tile([1, 4 * 128], F32, tag="mrow")
                for k in range(4):
                    nc.sync.dma_start(
                        mrow[:, k * 128 : (k + 1) * 128], mr_dram[k : k + 1, :])
                mu_bc = np1.tile([128, D], F32, tag="mubc")
                nc.gpsimd.partition_broadcast(mu_bc[:], mrow[:, 0:D])
                nc.gpsimd.partition_broadcast(rsd_bc[:], mrow[:, D : 2 * D])
                nc.vector.tensor_tensor(mrs_bc[:], mu_bc[:], rsd_bc[:], AL.mult)

            # ================ aggregation layers ================
            def agg_layer(lidx, tbl, row_b, row_dt, gat, nl, adst_sb, epilogue):
                """row_b = row bytes; nl = psf cols fed to epilogue."""
                elem = row_b // mybir.dt.size(row_dt)
                mmcols = 257 if gat else nl
                with (
                    tc.tile_pool(name=f"G{lidx}", bufs=4) as poolG,
                    tc.tile_pool(name=f"S{lidx}", bufs=4) as poolS,
                    tc.tile_pool(name=f"w{lidx}", bufs=4) as poolW,
                    tc.tile_pool(name=f"oh{lidx}", bufs=12) as poolOH,
                    tc.tile_pool(name=f"n{lidx}", bufs=3) as poolN,
                    tc.tile_pool(name=f"ep{lidx}", bufs=3) as poolE,
                    tc.tile_pool(name=f"pf{lidx}", bufs=2, space="PSUM") as poolPF,
                    tc.tile_pool(name=f"pc{lidx}", bufs=2, space="PSUM") as poolPC,
                    tc.tile_pool(name=f"px{lidx}", bufs=1, space="PSUM") as poolPX,
                    tc.tile_pool(name=f"pt{lidx}", bufs=2, space="PSUM") as poolPT,
                ):
                    G_tiles = {}
                    S_tiles = {}
                    W_tiles = {}

                    def get_G(st):
                        if st not in G_tiles:
                            g = poolG.tile([128, ST, elem], row_dt, tag="G")
                            base = (tbl[0:HALF, :] if st * ST < T_A_pad
                                    else tbl[HALF:NPAD, :])
                            nc.gpsimd.dma_gather(
                                g[:], base,
                                gidx_sb[:, st * (ST * 8) : (st + 1) * (ST * 8)],
                                ST * 128, ST * 128, elem,
                                single_packet=False, queue_num=st % GQ)
                            G_tiles[st] = g
                        return G_tiles[st]

                    def get_OHS(st):
                        if st not in S_tiles:
                            s = poolS.tile([128, ST * 128], FP8, tag="ohs")
                            nc.sync.dma_start(
                                s[:], ohg[:, st * ST * 128 : (st + 1) * ST * 128])
                            S_tiles[st] = s
                        return S_tiles[st]

                    def get_W(st):
                        if st not in W_tiles:
                            it = poolS.tile([128, ST * 128], FP8, tag="indt")
                            nc.sync.dma_start(
                                it[:], indTd[:, st * ST * 128 : (st + 1) * ST * 128])
                            ptc = poolPC.tile([128, ST], F32, tag="ptc")
                            for j in range(ST):
                                p = st * ST + j
                                w = int(chunk2win[p])
                                nc.tensor.matmul(
                                    ptc[:, j : j + 1],
                                    it[:, j * 128 : (j + 1) * 128],
                                    adst_sb[:, w : w + 1], start=True, stop=True)
                            g = get_G(st)
                            gb = g[:].bitcast(BF16)
                            tcs = poolN.tile([128, ST], F32, tag="tcs")
                            nc.scalar.activation(tcs[:], ptc[:], ACTF.Copy)
                            asr = poolN.tile([128, ST], F32, tag="asr")
                            nc.scalar.activation(asr[:], gb[:, :, 129], ACTF.Copy)
                            ell = poolN.tile([128, ST], F32, tag="ell")
                            nc.vector.tensor_tensor(ell[:], tcs[:], asr[:], AL.add)
                            e1 = poolN.tile([128, ST], BF16, tag="e1")
                            nc.scalar.activation(e1[:], ell[:], ACTF.Exp)
                            e5 = poolN.tile([128, ST], BF16, tag="e5")
                            nc.scalar.activation(e5[:], ell[:], ACTF.Exp, scale=NEG)
                            ws = poolW.tile([128, ST], F32, tag="ws")
                            nc.vector.tensor_tensor(ws[:], e1[:], e5[:], AL.max)
                            W_tiles[st] = ws
                        return W_tiles[st]

                    for w in range(NW):
                        chunks = win_chunks[w]
                        psf = poolPF.tile([128, mmcols], F32, tag="psf")
                        n = len(chunks)
                        if n == 0:
                            zps = poolE.tile([128, mmcols], F32, tag="zps")
                            nc.vector.memset(zps[:], 0.0)
                            epilogue(w, zps, (poolE, poolPX, poolPT))
                            continue
                        for i, p in enumerate(chunks):
                            st, j = p // ST, p % ST
                            g = get_G(st)
                            if gat:
                                ws = get_W(st)
                                oh = poolOH.tile([128, 128], BF16, tag="oh")
                                nc.vector.tensor_scalar(
                                    oh[:], iota_sb[:], dstc_sb[:, p : p + 1],
                                    ws[:, j : j + 1], AL.is_equal, AL.mult)
                            else:
                                ohs = get_OHS(st)
                                oh = ohs[:, j * 128 : (j + 1) * 128]
                            nc.tensor.matmul(
                                psf[:], oh if gat is False else oh[:],
                                g[:, j, 0:mmcols],
                                start=(i == 0), stop=(i == n - 1))
                        epilogue(w, psf, (poolE, poolPX, poolPT))

            # ---- epilogue helpers
            def transform_store(w, hb, rhs_sb, ncols, pools, store):
                poolE, poolPX, poolPT = pools
                px = poolPX.tile([128, ncols], F32, tag="px")
                for k in range(2):
                    pt = poolPT.tile([128, 128], BF16, tag="pt")
                    nc.tensor.transpose(
                        pt[:], hb[:, k * 128 : (k + 1) * 128], ident_sb[:])
                    ht = poolE.tile([128, 128], BF16, tag="ht")
                    nc.scalar.activation(ht[:], pt[:], ACTF.Copy)
                    nc.tensor.matmul(
                        px[:], ht[:], rhs_sb[:, k, 0:ncols],
                        start=(k == 0), stop=(k == 1))
                store(w, px)

            def store_gat(adst_next):
                def f(w, px):
                    nc.vector.tensor_copy(asm8[:, w, 0:256], px[:, 0:256])
                    nc.scalar.activation(
                        asm8_bf[:, w, 129:130], px[:, 256:257], ACTF.Copy)
                    nc.scalar.activation(
                        adst_next[:, w : w + 1], px[:, 257:258], ACTF.Copy)
                return f

            def store_l4(w, px):
                nc.scalar.activation(asm4[:, w, 0:64], px[:, 0:64], ACTF.Copy)

            def epi_l1(w, psf, pools):
                # agg_std = rsd*(agg_x + sln*x_own - mu*S_full); then the GCN
                # transform x1 = relu(agg_std @ W1 + b1), then @ Wg_ext.
                poolE, poolPX, poolPT = pools
                u0 = poolE.tile([128, D], F32, tag="u0")
                nc.vector.scalar_tensor_tensor(
                    u0[:], xown_sb[:, w, :], sln_sb[:, w : w + 1],
                    psf[:, 0:D], AL.mult, AL.add)
                ux = poolE.tile([128, D], F32, tag="ux")
                nc.vector.tensor_tensor(ux[:], u0[:], rsd_bc[:], AL.mult)
                hx = poolE.tile([128, D], BF16, tag="hx")
                nc.vector.scalar_tensor_tensor(
                    hx[:], mrs_bc[:], sfl_sb[:, w : w + 1], ux[:],
                    AL.mult, AL.add)
                px1 = poolPX.tile([128, D], F32, tag="px1")
                for k in range(2):
                    pt = poolPT.tile([128, 128], BF16, tag="pt")
                    nc.tensor.transpose(
                        pt[:], hx[:, k * 128 : (k + 1) * 128], ident_sb[:])
                    ht = poolE.tile([128, 128], BF16, tag="ht1")
                    nc.scalar.activation(ht[:], pt[:], ACTF.Copy)
                    nc.tensor.matmul(
                        px1[:], ht[:], W1_sb[:, k, :],
                        start=(k == 0), stop=(k == 1))
                lg1 = poolE.tile([128, D], F32, tag="lg1")
                nc.vector.scalar_tensor_tensor(
                    lg1[:], px1[:], 1.0, b1_sb[:], AL.mult, AL.add)
                hb = poolE.tile([128, D], BF16, tag="hb")
                nc.scalar.activation(hb[:], lg1[:], ACTF.Relu)
                transform_store(w, hb, Wg_sb, 258, pools, store_gat(adstA))

            def epi_gat(bias_sb, rhs_sb, ncols, store, adst_cur):
                def f(w, psf, pools):
                    poolE, _, _ = pools
                    # self-loop: w_self = exp(lrelu(asrc_d + adst_d))
                    lsf = poolE.tile([128, 1], F32, tag="lsf")
                    nc.vector.tensor_tensor(
                        lsf[:], asm8_bf[:, w, 129:130],
                        adst_cur[:, w : w + 1], AL.add)
                    e1s = poolE.tile([128, 1], F32, tag="e1s")
                    nc.scalar.activation(e1s[:], lsf[:], ACTF.Exp)
                    e5s = poolE.tile([128, 1], F32, tag="e5s")
                    nc.scalar.activation(e5s[:], lsf[:], ACTF.Exp, scale=NEG)
                    wse = poolE.tile([128, 1], F32, tag="wse")
                    nc.vector.tensor_tensor(wse[:], e1s[:], e5s[:], AL.max)
                    # z includes the self edge; z > 0 guaranteed
                    zt = poolE.tile([128, 1], F32, tag="zt")
                    nc.vector.tensor_scalar(
                        zt[:], psf[:, 256:257], 1e-30, None, AL.add)
                    zt2 = poolE.tile([128, 1], F32, tag="zt2")
                    nc.vector.tensor_tensor(zt2[:], zt[:], wse[:], AL.add)
                    rz = poolE.tile([128, 1], F32, tag="rz")
                    nc.vector.reciprocal(rz[:], zt2[:])
                    wrz = poolE.tile([128, 1], F32, tag="wrz")
                    nc.vector.tensor_tensor(wrz[:], wse[:], rz[:], AL.mult)
                    hs = poolE.tile([128, D], F32, tag="hs")
                    nc.vector.scalar_tensor_tensor(
                        hs[:], psf[:, 0:D], rz[:], bias_sb[:], AL.mult, AL.add)
                    # hs += (w_self/z) * own row
                    nc.vector.scalar_tensor_tensor(
                        hs[:], asm8[:, w, 0:D], wrz[:], hs[:], AL.mult, AL.add)
                    hb = poolE.tile([128, D], BF16, tag="hb")
                    nc.scalar.activation(hb[:], hs[:], ACTF.Relu)
                    transform_store(w, hb, rhs_sb, ncols, pools, store)
                return f

            def epi_l4(w, psf, pools):
                poolE, _, _ = pools
                lg = poolE.tile([128, 64], F32, tag="lg")
                nc.vector.scalar_tensor_tensor(
                    lg[:], psf[:, 0:64], 1.0, b2_sb[:], AL.mult, AL.add)
                nc.vector.scalar_tensor_tensor(
                    lg[:], asm4[:, w, 0:64], sln_sb[:, w : w + 1],
                    lg[:], AL.mult, AL.add)
                m = poolE.tile([128, 1], F32, tag="m")
                nc.vector.tensor_reduce(
                    m[:], lg[:, 0:LOUT], mybir.AxisListType.X, AL.max)
                nc.vector.tensor_scalar(
                    negm_all[:, w : w + 1], m[:], -1.0, None, AL.mult)
                es = poolE.tile([128, LOUT], F32, tag="es")
                nc.scalar.activation(
                    es[:], lg[:, 0:LOUT], ACTF.Exp,
                    bias=negm_all[:, w : w + 1], accum_out=z_all[:, w : w + 1])
                nc.vector.tensor_scalar(
                    out_asm[:, w, :], lg[:, 0:LOUT], negm_all[:, w : w + 1],
                    None, AL.add)

            # L1: GCN on T1
            if KSTOP >= 2:
                agg_layer(1, Xtbl, 256, FP8, gat=False, nl=D, adst_sb=None,
                          epilogue=epi_l1)
                nc.sync.dma_start(sh2[:].rearrange("w p c -> p w c"), asm8[:])
                nc.gpsimd.collective_compute(
                    "AllGather", AL.bypass, replica_groups=RG,
                    ins=[sh2[:].opt()], outs=[T2[:].opt()])

            # L2: GAT on T2
            if KSTOP >= 3:
                agg_layer(2, T2, 512, FP8, gat=True, nl=D, adst_sb=adstA,
                          epilogue=epi_gat(bg_sb, Wg_sb, 258, store_gat(adstB), adstA))
                nc.sync.dma_start(sh3[:].rearrange("w p c -> p w c"), asm8[:])
                nc.gpsimd.collective_compute(
                    "AllGather", AL.bypass, replica_groups=RG,
                    ins=[sh3[:].opt()], outs=[T3[:].opt()])

            # L3: GAT on T3
            if KSTOP >= 4:
                agg_layer(3, T3, 512, FP8, gat=True, nl=64, adst_sb=adstB,
                          epilogue=epi_gat(bg_sb, W2_sb, 64, store_l4, adstB))
                nc.sync.dma_start(sh4[:].rearrange("w p c -> p w c"), asm4[:])
                nc.gpsimd.collective_compute(
                    "AllGather", AL.bypass, replica_groups=RG,
                    ins=[sh4[:].opt()], outs=[T4[:].opt()])

            # L4: GCN on T4
            if KSTOP >= 5:
                agg_layer(4, T4, 256, BF16, gat=False, nl=64, adst_sb=None,
                          epilogue=epi_l4)
                lnz = pp.tile([128, NW], F32, tag="lnz")
                nc.scalar.activation(lnz[:], z_all[:], ACTF.Ln)
                for w in range(NW):
                    nc.vector.tensor_scalar(
                        out_asm[:, w, :], out_asm[:, w, :],
                        lnz[:, w : w + 1], None, AL.subtract)
            nc.sync.dma_start(out[:].rearrange("w p c -> p w c"), out_asm[:])

    nc.compile()
    return nc


# ---------------------------------------------------------------- entry

_CACHE = {}
_RUN_KWARGS = {}


def kernel(**inputs):
    edge_index = np.asarray(inputs["edge_index"])
    if "nc" not in _CACHE:
        meta, per_core = preprocess(edge_index)
        _CACHE["meta"] = meta
        _CACHE["per_core"] = per_core
        _CACHE["nc"] = build_nc(meta)
    nc = _CACHE["nc"]
    per_core = _CACHE["per_core"]

    wmaps = make_weight_inputs(inputs)
    x = np.asarray(inputs["x"], np.float32)
    xpad = np.zeros((NPAD, D), np.float32)
    xpad[:N] = x
    x8 = np.ascontiguousarray(xpad.astype(_F8))

    in_maps = []
    for r in range(NCORES):
        xs = xpad[r * SHARD : (r + 1) * SHARD].T
        m = dict(per_core[r])
        m.update(wmaps)
        m["xT"] = np.ascontiguousarray(xs.reshape(2, 128, SHARD))
        m["Xtbl"] = x8
        m["xown"] = np.ascontiguousarray(
            x8[r * SHARD : (r + 1) * SHARD].reshape(NW, 128, D)
            .transpose(1, 0, 2))
        in_maps.append(m)

    res = run_bass_kernel_spmd(nc, in_maps, core_ids=list(range(NCORES)), **_RUN_KWARGS)
    _CACHE["last_res"] = res
    outs = [r["out"].reshape(SHARD, LOUT) for r in res.results]
    full = np.concatenate(outs, 0)[:N]
    return full.astype(np.float32)


if __name__ == "__main__":
    import reference

    inputs = {k: np.asarray(v) for k, v in reference.setup_inputs().items()}
    got = kernel(**inputs)
    print("kernel output", got.shape, got.dtype)
